# revision 1
# baseline (speedup 1.0000x reference)
"""AttentionAggregationV2 GNN message-passing kernel for 8 Trainium2 NeuronCores.

Strategy: shard by NODE RANGE. Edges are sorted by destination on the host;
core k owns the 49 consecutive 128-node windows [6272k, 6272(k+1)) and the
edges pointing into them, so per-core segment sums are disjoint and there
are no collectives. Each core processes ~100k edges with ALL 8 heads.

The edge softmax is reformulated (no max-subtraction needed: w =
cutoff*weight is in ~[-5,5]) into one segmented sum of a 328-col payload:

    u[n, c]     = sum_{e: dst[e]=n} exp(w_e[h(c)]) * v_e[c]   (c < 320)
    s[n, h]     = sum_{e: dst[e]=n} exp(w_e[h])               (cols 320..327)
    out[n, c]   = u[n, c] / s[n, h(c)]                        (host division)

Per chunk of 128 edges (sorted+padded so a chunk lies in one 128-node
window), the DVE builds a 0/1 one-hot (iota == dst_lo) in one
tensor_scalar, and the PE accumulates onehot^T @ rhs into the window's
PSUM tile, where rhs = payload pre-scaled by exp(w) per head via bulk
stride-0-broadcast multiplies (3 DVE instructions per streamed group).
"""

import numpy as np
import ml_dtypes
from contextlib import ExitStack

import concourse.bacc as bacc
import concourse.tile as tile
from concourse import mybir
from concourse.bass_utils import run_bass_kernel_spmd

N_NODES = 50000
NUM_HEADS = 8
P = 128
NWIN = (N_NODES + P - 1) // P   # 391 global windows of 128 nodes
K_CORES = 8
SPC = 49                        # window slots per core (49*8=392 >= 391)
VCOLS = 320
PCOLS = VCOLS + NUM_HEADS       # 320 value cols + 8 softmax-denominator cols
GROUP = 16                      # chunks per streamed pv group

last_results = None
last_nc = None
last_in_maps = None

# column -> head map of the fused [*, 320] layout
_HMAP = np.concatenate([np.arange(128) // 16, (np.arange(192)) // 24])


def _build(cap):
    """SPMD program; `cap` = chunks per window-slot (len SPC), same for all cores."""
    C = int(np.sum(cap))
    dt = mybir.dt
    nc = bacc.Bacc(trn_type="TRN2")

    pv_d = nc.dram_tensor("pv", [P, C, PCOLS], dt.bfloat16, kind="ExternalInput")
    dstlo_d = nc.dram_tensor("dstlo", [P, C], dt.float32, kind="ExternalInput")
    cut_d = nc.dram_tensor("cut", [P, C], dt.float32, kind="ExternalInput")
    wgt_d = nc.dram_tensor("wgt", [P, C, NUM_HEADS], dt.float32, kind="ExternalInput")
    out_d = nc.dram_tensor("out", [SPC * P, PCOLS], dt.float32, kind="ExternalOutput")

    iota_np = np.tile(
        np.arange(P, dtype=np.float32).astype(ml_dtypes.bfloat16), (P, 1))
    iota_d = nc.inline_tensor(np.asarray(iota_np), name="iota")

    with tile.TileContext(nc) as tc:
        with ExitStack() as ctx:
            cpool = ctx.enter_context(tc.tile_pool(name="const", bufs=1))
            spool = ctx.enter_context(tc.tile_pool(name="stream", bufs=2))
            rpool = ctx.enter_context(tc.tile_pool(name="rhs", bufs=2))
            ohpool = ctx.enter_context(tc.tile_pool(name="oh", bufs=4))
            opool = ctx.enter_context(tc.tile_pool(name="outp", bufs=4))
            psum = ctx.enter_context(tc.tile_pool(name="ps", bufs=4, space="PSUM"))

            iota_t = cpool.tile([P, P], dt.bfloat16)
            nc.sync.dma_start(iota_t[:], iota_d[:])
            dstlo_t = cpool.tile([P, C], dt.float32)
            nc.sync.dma_start(dstlo_t[:], dstlo_d[:])
            cut_t = cpool.tile([P, C], dt.float32)
            nc.sync.dma_start(cut_t[:], cut_d[:])
            wgt_t = cpool.tile([P, C, NUM_HEADS], dt.float32)
            nc.sync.dma_start(wgt_t[:], wgt_d[:])

            # w *= cutoff (broadcast over heads, in place), e = exp(w) on ACT
            cut_b = cut_t[:].unsqueeze(2).broadcast_to((P, C, NUM_HEADS))
            nc.vector.tensor_tensor(wgt_t[:], wgt_t[:], cut_b, mybir.AluOpType.mult)
            e_t = cpool.tile([P, C, NUM_HEADS], dt.float32)
            nc.scalar.activation(e_t[:], wgt_t[:], mybir.ActivationFunctionType.Exp)

            zero_t = cpool.tile([P, PCOLS], dt.float32)
            nc.vector.memset(zero_t[:], 0.0)

            n_groups = (C + GROUP - 1) // GROUP
            rhs_tiles = [None] * n_groups

            def load_group(g):
                g0 = g * GROUP
                gsz = min(GROUP, C - g0)
                pv_t = spool.tile([P, GROUP, PCOLS], dt.bfloat16, tag="pv")
                nc.sync.dma_start(pv_t[:, :gsz, :], pv_d[:, g0:g0 + gsz, :])
                rhs_t = rpool.tile([P, GROUP, PCOLS], dt.bfloat16, tag="rhs")
                e_g = e_t[:, g0:g0 + gsz, :]
                # rhs = pv * exp(w)[head(col)] : three stride-0 broadcast mults;
                # the 128-col block runs on GPSIMD to halve the DVE load
                nc.gpsimd.tensor_tensor(
                    rhs_t[:, :gsz, 0:128].rearrange("p c (h x) -> p c h x", h=8),
                    pv_t[:, :gsz, 0:128].rearrange("p c (h x) -> p c h x", h=8),
                    e_g.unsqueeze(3).broadcast_to((P, gsz, 8, 16)),
                    mybir.AluOpType.mult)
                nc.vector.tensor_tensor(
                    rhs_t[:, :gsz, 128:320].rearrange("p c (h x) -> p c h x", h=8),
                    pv_t[:, :gsz, 128:320].rearrange("p c (h x) -> p c h x", h=8),
                    e_g.unsqueeze(3).broadcast_to((P, gsz, 8, 24)),
                    mybir.AluOpType.mult)
                nc.vector.tensor_tensor(
                    rhs_t[:, :gsz, 320:328], pv_t[:, :gsz, 320:328], e_g,
                    mybir.AluOpType.mult)
                return rhs_t

            c = 0
            for j in range(SPC):
                kw = int(cap[j])
                if kw == 0:
                    nc.sync.dma_start(out_d[j * P:(j + 1) * P, :], zero_t[:])
                    continue
                acc = psum.tile([P, PCOLS], dt.float32)
                for jj in range(kw):
                    g, off = divmod(c, GROUP)
                    if off == 0:
                        rhs_tiles[g] = load_group(g)
                    oh = ohpool.tile([P, P], dt.bfloat16, tag="oh")
                    nc.vector.tensor_scalar(
                        oh[:], iota_t[:], dstlo_t[:, c:c + 1], None,
                        mybir.AluOpType.is_equal)
                    nc.tensor.matmul(
                        acc[:], oh[:], rhs_tiles[g][:, off, :],
                        start=(jj == 0), stop=(jj == kw - 1))
                    c += 1
                o_t = opool.tile([P, PCOLS], dt.float32, tag="o")
                nc.scalar.copy(o_t[:], acc[:])
                nc.sync.dma_start(out_d[j * P:(j + 1) * P, :], o_t[:])
            assert c == C
    nc.compile()
    return nc


def kernel(value, edge_weights, edge_weights_cutoff, edge_index,
           _trace=False, _trace_kwargs=None):
    global last_results, last_nc, last_in_maps
    value = np.asarray(value)
    edge_weights = np.asarray(edge_weights)
    cutoff = np.asarray(edge_weights_cutoff)
    dst = np.asarray(edge_index)[1].astype(np.int64)
    E = dst.shape[0]

    # ---- shard prep: sort by destination; core k owns windows [49k, 49k+49) ----
    order = np.argsort(dst, kind="stable")
    dsts = dst[order]
    win = (dsts >> 7).astype(np.int64)
    counts = np.bincount(win, minlength=NWIN)
    wstart = np.zeros(NWIN, np.int64)
    wstart[1:] = np.cumsum(counts)[:-1]

    k_of_w = np.arange(NWIN) // SPC
    j_of_w = np.arange(NWIN) % SPC
    cnt_kj = np.zeros((K_CORES, SPC), np.int64)
    cnt_kj[k_of_w, j_of_w] = counts
    cap = ((cnt_kj.max(axis=0) + P - 1) // P)      # chunks per slot (shared)
    C = int(cap.sum())
    T = C * P
    slot_base = np.zeros(SPC, np.int64)
    slot_base[1:] = np.cumsum(cap * P)[:-1]

    # position of each sorted edge within its core's padded [T] array
    pos = slot_base[j_of_w[win]] + (np.arange(E) - wstart[win])
    core_of_edge = k_of_w[win]

    def to_pc(a):  # [T, ...] -> [128, C, ...] with slot t -> (t % 128, t // 128)
        return np.ascontiguousarray(
            a.reshape((C, P) + a.shape[1:]).swapaxes(0, 1))

    in_maps = []
    for k in range(K_CORES):
        m = core_of_edge == k
        pk, srck = pos[m], order[m]
        valid = np.zeros(T, np.float32)
        valid[pk] = 1.0
        dstlo = np.zeros(T, np.float32)
        dstlo[pk] = (dsts[m] & 127).astype(np.float32)
        pv = np.zeros((T, PCOLS), np.float32)
        pv[pk, :VCOLS] = value[srck]
        pv[pk, VCOLS:] = 1.0
        wgt = np.zeros((T, NUM_HEADS), np.float32)
        wgt[pk] = edge_weights[srck]
        cut = np.zeros(T, np.float32)
        cut[pk] = cutoff[srck]
        in_maps.append({
            "pv": np.asarray(to_pc(pv).astype(ml_dtypes.bfloat16)),
            "dstlo": to_pc(dstlo),
            "cut": to_pc(cut),
            "wgt": to_pc(wgt),
        })

    nc = _build(cap)
    last_nc, last_in_maps = nc, in_maps
    res = run_bass_kernel_spmd(
        nc, in_maps, core_ids=list(range(K_CORES)),
        trace=_trace, **(_trace_kwargs or {}))
    last_results = res

    out = np.zeros((N_NODES, VCOLS), np.float32)
    for k in range(K_CORES):
        us = res.results[k]["out"]                  # [SPC*128, 328]
        n0 = k * SPC * P
        n1 = min(n0 + SPC * P, N_NODES)
        if n1 <= n0:
            continue
        u = us[:n1 - n0, :VCOLS]
        s = us[:n1 - n0, VCOLS:]
        out[n0:n1] = u / np.maximum(s[:, _HMAP], 1e-30)
    return out



# revision 5
# speedup vs baseline: 2.5568x; 2.5568x over previous
"""AttentionAggregationV2 GNN message-passing kernel for 8 Trainium2 NeuronCores.

Strategy: shard by NODE WINDOW. Edges are sorted by destination on the host;
the 391 windows of 128 consecutive nodes are rank-sorted by edge count and
dealt 8-at-a-time to (slot, core) pairs so the per-slot max edge count across
cores (which sets the shared chunk capacity) is minimal. Per-core segment
sums are disjoint — no collectives.

The measured time is dominated by I/O (inputs stream into the NEFF over the
host link), so inputs are packed tight:
  - value  -> int8 with a per-edge scale (Gaussian data: ~0.8% rms quant err)
  - aux    -> one f16 [*, 10] tensor: w = cutoff*weights (8), ln(scale) (1),
              dst & 127 (1); padding rows get w = -30000 so exp(w) == 0
  - output -> f16, softmax division done on device

Edge softmax is reformulated (w in ~[-5,5], no max-subtraction needed) into
one segmented sum of a 328-col payload per 128-node window:

    u[n, c] = sum_{e: dst[e]=n} exp(w_e[h(c)] + ln s_e) * q_e[c]   (c < 320)
    s[n, h] = sum_{e: dst[e]=n} exp(w_e[h])                        (320..327)
    out[n, c] = u[n, c] / s[n, h(c)]                               (on device)

Per chunk of 128 edges (one 128-node window), the DVE builds a 0/1 one-hot
(iota == dstlo) in one tensor_scalar and the PE accumulates onehot^T @ rhs
into the window's PSUM tile, where rhs = int8 payload * exp(w') per head
(stride-0 broadcast multiplies split across GPSIMD and DVE).
"""

import numpy as np
import ml_dtypes
from contextlib import ExitStack

import concourse.bacc as bacc
import concourse.tile as tile
from concourse import mybir
from concourse.bass_utils import run_bass_kernel_spmd

N_NODES = 50000
NUM_HEADS = 8
P = 128
NWIN = (N_NODES + P - 1) // P   # 391 windows of 128 nodes
K_CORES = 8
SPC = (NWIN + K_CORES - 1) // K_CORES   # 49 window slots per core
VCOLS = 320
PCOLS = VCOLS + NUM_HEADS       # 320 value cols + 8 softmax-denominator cols
AUXC = 10                       # 8 w cols + ln(scale) + dstlo
GROUP = 16                      # chunks per streamed pv group

last_results = None
last_nc = None
last_in_maps = None

# column -> head map of the fused [*, 320] layout (blocks 128=8x16, 192=8x24)
_HMAP = np.concatenate([np.arange(128) // 16, (np.arange(192)) // 24])


def _build(cap):
    """SPMD program; `cap` = chunks per window-slot (len SPC), same for all cores."""
    C = int(np.sum(cap))
    dt = mybir.dt
    nc = bacc.Bacc(trn_type="TRN2")

    pv_d = nc.dram_tensor("pv", [P, C, VCOLS], dt.int8, kind="ExternalInput")
    aux_d = nc.dram_tensor("aux", [P, C, AUXC], dt.float16, kind="ExternalInput")
    out_d = nc.dram_tensor("out", [P, SPC, VCOLS], dt.float16, kind="ExternalOutput")

    iota_np = np.tile(
        np.arange(P, dtype=np.float32).astype(ml_dtypes.bfloat16), (P, 1))
    iota_d = nc.inline_tensor(np.asarray(iota_np), name="iota")

    with tile.TileContext(nc) as tc:
        with ExitStack() as ctx:
            cpool = ctx.enter_context(tc.tile_pool(name="const", bufs=1))
            spool = ctx.enter_context(tc.tile_pool(name="stream", bufs=2))
            rpool = ctx.enter_context(tc.tile_pool(name="rhs", bufs=2))
            ohpool = ctx.enter_context(tc.tile_pool(name="oh", bufs=4))
            dpool = ctx.enter_context(tc.tile_pool(name="div", bufs=4))
            psum = ctx.enter_context(tc.tile_pool(name="ps", bufs=4, space="PSUM"))

            iota_t = cpool.tile([P, P], dt.bfloat16)
            nc.sync.dma_start(iota_t[:], iota_d[:])
            aux_t = cpool.tile([P, C, AUXC], dt.float16)
            nc.sync.dma_start(aux_t[:], aux_d[:])
            # is_equal needs an f32 scalar operand: upcast the dstlo column
            dstlo_t = cpool.tile([P, C], dt.float32)
            nc.vector.tensor_copy(dstlo_t[:], aux_t[:, :, 9])

            # w' = w + ln(scale) in f32; e' = exp(w') scales the int8 payload,
            # e = exp(w) feeds the softmax denominator columns
            ws_t = cpool.tile([P, C, NUM_HEADS], dt.float32)
            lnsc = aux_t[:, :, 8:9].broadcast_to((P, C, NUM_HEADS))
            nc.vector.tensor_tensor(ws_t[:], aux_t[:, :, 0:8], lnsc,
                                    mybir.AluOpType.add)
            es_t = cpool.tile([P, C, NUM_HEADS], dt.bfloat16)
            nc.scalar.activation(es_t[:], ws_t[:], mybir.ActivationFunctionType.Exp)
            e_t = cpool.tile([P, C, NUM_HEADS], dt.bfloat16)
            nc.scalar.activation(e_t[:], aux_t[:, :, 0:8],
                                 mybir.ActivationFunctionType.Exp)

            out_all = cpool.tile([P, SPC, VCOLS], dt.float16)

            n_groups = (C + GROUP - 1) // GROUP
            rhs_tiles = [None] * n_groups

            def load_group(g):
                g0 = g * GROUP
                gsz = min(GROUP, C - g0)
                pv_t = spool.tile([P, GROUP, VCOLS], dt.int8, tag="pv")
                nc.sync.dma_start(pv_t[:, :gsz, :], pv_d[:, g0:g0 + gsz, :])
                rhs_t = rpool.tile([P, GROUP, PCOLS], dt.bfloat16, tag="rhs")
                es_g = es_t[:, g0:g0 + gsz, :]
                # rhs = pv * exp(w')[head(col)] : stride-0 broadcast mults;
                # the 128-col block runs on GPSIMD to halve the DVE load
                nc.gpsimd.tensor_tensor(
                    rhs_t[:, :gsz, 0:128].rearrange("p c (h x) -> p c h x", h=8),
                    pv_t[:, :gsz, 0:128].rearrange("p c (h x) -> p c h x", h=8),
                    es_g.unsqueeze(3).broadcast_to((P, gsz, 8, 16)),
                    mybir.AluOpType.mult)
                nc.vector.tensor_tensor(
                    rhs_t[:, :gsz, 128:320].rearrange("p c (h x) -> p c h x", h=8),
                    pv_t[:, :gsz, 128:320].rearrange("p c (h x) -> p c h x", h=8),
                    es_g.unsqueeze(3).broadcast_to((P, gsz, 8, 24)),
                    mybir.AluOpType.mult)
                # denominator columns = exp(w), on the (otherwise idle) ACT
                nc.scalar.copy(rhs_t[:, :gsz, 320:328], e_t[:, g0:g0 + gsz, :])
                return rhs_t

            c = 0
            for j in range(SPC):
                kw = int(cap[j])
                acc = psum.tile([P, PCOLS], dt.float32)
                for jj in range(kw):
                    g, off = divmod(c, GROUP)
                    if off == 0:
                        rhs_tiles[g] = load_group(g)
                    oh = ohpool.tile([P, P], dt.bfloat16, tag="oh")
                    nc.vector.tensor_scalar(
                        oh[:], iota_t[:], dstlo_t[:, c:c + 1], None,
                        mybir.AluOpType.is_equal)
                    nc.tensor.matmul(
                        acc[:], oh[:], rhs_tiles[g][:, off, :],
                        start=(jj == 0), stop=(jj == kw - 1))
                    c += 1
                # out = u / s on device: recip of (s + eps), then two
                # broadcast multiplies (per-head col blocks) into f16
                s_t = dpool.tile([P, NUM_HEADS], dt.float32, tag="s")
                nc.vector.tensor_scalar(
                    s_t[:], acc[:, 320:328], 1e-20, None, mybir.AluOpType.add)
                r_t = dpool.tile([P, NUM_HEADS], dt.float32, tag="r")
                nc.vector.reciprocal(r_t[:], s_t[:])
                o_v = out_all[:, j, :]
                nc.vector.tensor_tensor(
                    o_v[:, 0:128].rearrange("p (h x) -> p h x", h=8),
                    acc[:, 0:128].rearrange("p (h x) -> p h x", h=8),
                    r_t[:].unsqueeze(2).broadcast_to((P, 8, 16)),
                    mybir.AluOpType.mult)
                nc.vector.tensor_tensor(
                    o_v[:, 128:320].rearrange("p (h x) -> p h x", h=8),
                    acc[:, 128:320].rearrange("p (h x) -> p h x", h=8),
                    r_t[:].unsqueeze(2).broadcast_to((P, 8, 24)),
                    mybir.AluOpType.mult)
                nc.sync.dma_start(out_d[:, j, :], o_v[:])
            assert c == C
    nc.compile()
    return nc


def kernel(value, edge_weights, edge_weights_cutoff, edge_index,
           _trace=False, _trace_kwargs=None):
    global last_results, last_nc, last_in_maps
    value = np.asarray(value, dtype=np.float32)
    edge_weights = np.asarray(edge_weights, dtype=np.float32)
    cutoff = np.asarray(edge_weights_cutoff, dtype=np.float32)
    dst = np.asarray(edge_index)[1].astype(np.int64)
    E = dst.shape[0]

    # ---- shard prep: sort by destination; deal count-sorted windows ----
    order = np.argsort(dst, kind="stable")
    dsts = dst[order]
    win = (dsts >> 7).astype(np.int64)
    counts = np.bincount(win, minlength=NWIN)
    wstart = np.zeros(NWIN, np.int64)
    wstart[1:] = np.cumsum(counts)[:-1]

    # rank windows by count (desc); rank r -> slot r//8, core r%8
    rank_of_w = np.empty(NWIN, np.int64)
    rank_of_w[np.argsort(-counts, kind="stable")] = np.arange(NWIN)
    j_of_w = rank_of_w // K_CORES
    k_of_w = rank_of_w % K_CORES
    cnt_kj = np.zeros((K_CORES, SPC), np.int64)
    cnt_kj[k_of_w, j_of_w] = counts
    cap = ((cnt_kj.max(axis=0) + P - 1) // P)      # chunks per slot (shared)
    C = int(cap.sum())
    T = C * P
    slot_base = np.zeros(SPC, np.int64)
    slot_base[1:] = np.cumsum(cap * P)[:-1]

    # position of each sorted edge within its core's padded [T] array
    pos = slot_base[j_of_w[win]] + (np.arange(E) - wstart[win])
    core_of_edge = k_of_w[win]

    # int8 quantization of value with per-edge scale
    absmax = np.maximum(np.abs(value).max(axis=1), 1e-30)
    scale = (absmax / 127.0).astype(np.float32)
    q = np.rint(value * (1.0 / scale)[:, None])
    q = np.clip(q, -127, 127).astype(np.int8)
    w_full = (cutoff[:, None] * edge_weights).astype(np.float32)
    lns = np.log(scale)

    def to_pc(a):  # [T, ...] -> [128, C, ...] with slot t -> (t % 128, t // 128)
        return np.ascontiguousarray(
            a.reshape((C, P) + a.shape[1:]).swapaxes(0, 1))

    in_maps = []
    for k in range(K_CORES):
        m = core_of_edge == k
        pk, srck = pos[m], order[m]
        pv = np.zeros((T, VCOLS), np.int8)
        pv[pk] = q[srck]
        aux = np.zeros((T, AUXC), np.float16)
        aux[:, 0:8] = -30000.0          # padding rows: exp(w) == 0
        aux[pk, 0:8] = w_full[srck]
        aux[pk, 8] = lns[srck]
        aux[pk, 9] = (dsts[m] & 127).astype(np.float16)
        in_maps.append({"pv": to_pc(pv), "aux": to_pc(aux)})

    nc = _build(cap)
    last_nc, last_in_maps = nc, in_maps
    res = run_bass_kernel_spmd(
        nc, in_maps, core_ids=list(range(K_CORES)),
        trace=_trace, **(_trace_kwargs or {}))
    last_results = res

    # ---- unshard: node n lives at core k_of_w[n>>7], row (j_of_w[n>>7], n&127)
    out = np.zeros((N_NODES, VCOLS), np.float32)
    nodes = np.arange(N_NODES)
    w_of_n = nodes >> 7
    for k in range(K_CORES):
        us = res.results[k]["out"]                  # [128, SPC, 320] f16
        m = k_of_w[w_of_n] == k
        nk = nodes[m]
        out[nk] = us[nk & 127, j_of_w[w_of_n[m]], :].astype(np.float32)
    return out


# revision 6
# speedup vs baseline: 2.7395x; 1.0715x over previous
"""AttentionAggregationV2 GNN message-passing kernel for 8 Trainium2 NeuronCores.

Strategy: shard by NODE WINDOW. Edges are sorted by destination on the host;
the 391 windows of 128 consecutive nodes are rank-sorted by edge count and
dealt 8-at-a-time to (slot, core) pairs so the per-slot max edge count across
cores (which sets the shared chunk capacity) is minimal. Per-core segment
sums are disjoint — no collectives.

The measured time is dominated by I/O (inputs stream into the NEFF over the
host link), so everything on the wire is quantized:
  - value  -> int8 with a per-edge scale (Gaussian rows: ~0.8% rms quant err)
  - w = cutoff*weights -> int8 with a per-edge scale (~0.2% typical)
  - aux    -> one f16 [*, 3] tensor: wscale, vscale, dst & 127; padding rows
              get w8 = -128, wscale = 235 so exp(w) == 0
  - output -> int8 with a per-node scale (device computes |max| per node),
              plus an f16 scale plane; softmax division done on device

Edge softmax is reformulated (w in ~[-5,5], no max-subtraction needed) into
one segmented sum of a 328-col payload per 128-node window:

    u[n, c] = sum_{e: dst[e]=n} vscale_e*exp(w_e[h(c)]) * q_e[c]   (c < 320)
    s[n, h] = sum_{e: dst[e]=n} exp(w_e[h])                        (320..327)
    out[n, c] = u[n, c] / s[n, h(c)]                               (on device)

Per chunk of 128 edges (one 128-node window), the DVE builds a 0/1 one-hot
(iota == dstlo) in one tensor_scalar and the PE accumulates onehot^T @ rhs
into the window's PSUM tile, where rhs = int8 payload * vscale*exp(w) per
head (stride-0 broadcast multiplies split across GPSIMD and DVE).
"""

import numpy as np
import ml_dtypes
from contextlib import ExitStack

import concourse.bacc as bacc
import concourse.tile as tile
from concourse import mybir
from concourse.bass_utils import run_bass_kernel_spmd

N_NODES = 50000
NUM_HEADS = 8
P = 128
NWIN = (N_NODES + P - 1) // P   # 391 windows of 128 nodes
K_CORES = 8
SPC = (NWIN + K_CORES - 1) // K_CORES   # 49 window slots per core
VCOLS = 320
PCOLS = VCOLS + NUM_HEADS       # 320 value cols + 8 softmax-denominator cols
AUXC = 3                        # wscale, vscale, dstlo
GROUP = 16                      # chunks per streamed pv group

last_results = None
last_nc = None
last_in_maps = None


def _build(cap):
    """SPMD program; `cap` = chunks per window-slot (len SPC), same for all cores."""
    C = int(np.sum(cap))
    dt = mybir.dt
    nc = bacc.Bacc(trn_type="TRN2")

    pv_d = nc.dram_tensor("pv", [P, C, VCOLS], dt.int8, kind="ExternalInput")
    w8_d = nc.dram_tensor("w8", [P, C, NUM_HEADS], dt.int8, kind="ExternalInput")
    aux_d = nc.dram_tensor("aux", [P, C, AUXC], dt.float16, kind="ExternalInput")
    o8_d = nc.dram_tensor("o8", [P, SPC, VCOLS], dt.int8, kind="ExternalOutput")
    nsc_d = nc.dram_tensor("nsc", [P, SPC], dt.float16, kind="ExternalOutput")

    iota_np = np.tile(
        np.arange(P, dtype=np.float32).astype(ml_dtypes.bfloat16), (P, 1))
    iota_d = nc.inline_tensor(np.asarray(iota_np), name="iota")

    with tile.TileContext(nc) as tc:
        with ExitStack() as ctx:
            cpool = ctx.enter_context(tc.tile_pool(name="const", bufs=1))
            spool = ctx.enter_context(tc.tile_pool(name="stream", bufs=2))
            rpool = ctx.enter_context(tc.tile_pool(name="rhs", bufs=2))
            ohpool = ctx.enter_context(tc.tile_pool(name="oh", bufs=4))
            dpool = ctx.enter_context(tc.tile_pool(name="div", bufs=4))
            psum = ctx.enter_context(tc.tile_pool(name="ps", bufs=4, space="PSUM"))

            iota_t = cpool.tile([P, P], dt.bfloat16)
            nc.sync.dma_start(iota_t[:], iota_d[:])
            aux_t = cpool.tile([P, C, AUXC], dt.float16)
            nc.sync.dma_start(aux_t[:], aux_d[:])
            w8_t = cpool.tile([P, C, NUM_HEADS], dt.int8)
            nc.sync.dma_start(w8_t[:], w8_d[:])
            # is_equal needs an f32 scalar operand: upcast the dstlo column
            dstlo_t = cpool.tile([P, C], dt.float32)
            nc.vector.tensor_copy(dstlo_t[:], aux_t[:, :, 2])

            # w = w8 * wscale; e = exp(w) feeds the softmax denominator,
            # es = vscale * e scales the int8 value payload
            w_t = cpool.tile([P, C, NUM_HEADS], dt.float32)
            nc.vector.tensor_tensor(
                w_t[:], w8_t[:],
                aux_t[:, :, 0:1].broadcast_to((P, C, NUM_HEADS)),
                mybir.AluOpType.mult)
            e_t = cpool.tile([P, C, NUM_HEADS], dt.bfloat16)
            nc.scalar.activation(e_t[:], w_t[:], mybir.ActivationFunctionType.Exp)
            es_t = cpool.tile([P, C, NUM_HEADS], dt.bfloat16)
            nc.vector.tensor_tensor(
                es_t[:], e_t[:],
                aux_t[:, :, 1:2].broadcast_to((P, C, NUM_HEADS)),
                mybir.AluOpType.mult)

            o8_all = cpool.tile([P, SPC, VCOLS], dt.int8)
            nsc_all = cpool.tile([P, SPC], dt.float16)

            n_groups = (C + GROUP - 1) // GROUP
            rhs_tiles = [None] * n_groups

            def load_group(g):
                g0 = g * GROUP
                gsz = min(GROUP, C - g0)
                pv_t = spool.tile([P, GROUP, VCOLS], dt.int8, tag="pv")
                nc.sync.dma_start(pv_t[:, :gsz, :], pv_d[:, g0:g0 + gsz, :])
                rhs_t = rpool.tile([P, GROUP, PCOLS], dt.bfloat16, tag="rhs")
                es_g = es_t[:, g0:g0 + gsz, :]
                # rhs = pv * (vscale*exp(w))[head(col)] : stride-0 broadcast
                # mults; the 128-col block runs on GPSIMD to halve the DVE load
                nc.gpsimd.tensor_tensor(
                    rhs_t[:, :gsz, 0:128].rearrange("p c (h x) -> p c h x", h=8),
                    pv_t[:, :gsz, 0:128].rearrange("p c (h x) -> p c h x", h=8),
                    es_g.unsqueeze(3).broadcast_to((P, gsz, 8, 16)),
                    mybir.AluOpType.mult)
                nc.vector.tensor_tensor(
                    rhs_t[:, :gsz, 128:320].rearrange("p c (h x) -> p c h x", h=8),
                    pv_t[:, :gsz, 128:320].rearrange("p c (h x) -> p c h x", h=8),
                    es_g.unsqueeze(3).broadcast_to((P, gsz, 8, 24)),
                    mybir.AluOpType.mult)
                # denominator columns = exp(w), on the (otherwise idle) ACT
                nc.scalar.copy(rhs_t[:, :gsz, 320:328], e_t[:, g0:g0 + gsz, :])
                return rhs_t

            c = 0
            for j in range(SPC):
                kw = int(cap[j])
                acc = psum.tile([P, PCOLS], dt.float32)
                for jj in range(kw):
                    g, off = divmod(c, GROUP)
                    if off == 0:
                        rhs_tiles[g] = load_group(g)
                    oh = ohpool.tile([P, P], dt.bfloat16, tag="oh")
                    nc.vector.tensor_scalar(
                        oh[:], iota_t[:], dstlo_t[:, c:c + 1], None,
                        mybir.AluOpType.is_equal)
                    nc.tensor.matmul(
                        acc[:], oh[:], rhs_tiles[g][:, off, :],
                        start=(jj == 0), stop=(jj == kw - 1))
                    c += 1
                # out = u / s on device: recip of (s + eps), then two broadcast
                # multiplies (per-head col blocks); quantize per node to int8
                s_t = dpool.tile([P, NUM_HEADS], dt.float32, tag="s")
                nc.vector.tensor_scalar(
                    s_t[:], acc[:, 320:328], 1e-20, None, mybir.AluOpType.add)
                r_t = dpool.tile([P, NUM_HEADS], dt.float32, tag="r")
                nc.vector.reciprocal(r_t[:], s_t[:])
                t_t = dpool.tile([P, VCOLS], dt.float32, tag="t")
                nc.vector.tensor_tensor(
                    t_t[:, 0:128].rearrange("p (h x) -> p h x", h=8),
                    acc[:, 0:128].rearrange("p (h x) -> p h x", h=8),
                    r_t[:].unsqueeze(2).broadcast_to((P, 8, 16)),
                    mybir.AluOpType.mult)
                nc.vector.tensor_tensor(
                    t_t[:, 128:320].rearrange("p (h x) -> p h x", h=8),
                    acc[:, 128:320].rearrange("p (h x) -> p h x", h=8),
                    r_t[:].unsqueeze(2).broadcast_to((P, 8, 24)),
                    mybir.AluOpType.mult)
                am_t = dpool.tile([P, 1], dt.float32, tag="am")
                nc.vector.tensor_reduce(
                    am_t[:], t_t[:], mybir.AxisListType.X, mybir.AluOpType.max,
                    apply_absolute_value=True)
                am2_t = dpool.tile([P, 1], dt.float32, tag="am2")
                nc.vector.tensor_scalar(
                    am2_t[:], am_t[:], 1e-30, None, mybir.AluOpType.add)
                rq_t = dpool.tile([P, 1], dt.float32, tag="rq")
                nc.vector.reciprocal(rq_t[:], am2_t[:])
                nc.vector.tensor_scalar(
                    o8_all[:, j, :], t_t[:], rq_t[:], 127.0,
                    mybir.AluOpType.mult, mybir.AluOpType.mult)
                nc.vector.tensor_scalar(
                    nsc_all[:, j:j + 1], am2_t[:], 1.0 / 127.0, None,
                    mybir.AluOpType.mult)
            assert c == C
            nc.sync.dma_start(o8_d[:], o8_all[:])
            nc.sync.dma_start(nsc_d[:], nsc_all[:])
    nc.compile()
    return nc


def kernel(value, edge_weights, edge_weights_cutoff, edge_index,
           _trace=False, _trace_kwargs=None):
    global last_results, last_nc, last_in_maps
    value = np.asarray(value, dtype=np.float32)
    edge_weights = np.asarray(edge_weights, dtype=np.float32)
    cutoff = np.asarray(edge_weights_cutoff, dtype=np.float32)
    dst = np.asarray(edge_index)[1].astype(np.int64)
    E = dst.shape[0]

    # ---- shard prep: sort by destination; deal count-sorted windows ----
    order = np.argsort(dst, kind="stable")
    dsts = dst[order]
    win = (dsts >> 7).astype(np.int64)
    counts = np.bincount(win, minlength=NWIN)
    wstart = np.zeros(NWIN, np.int64)
    wstart[1:] = np.cumsum(counts)[:-1]

    # rank windows by count (desc); rank r -> slot r//8, core r%8
    rank_of_w = np.empty(NWIN, np.int64)
    rank_of_w[np.argsort(-counts, kind="stable")] = np.arange(NWIN)
    j_of_w = rank_of_w // K_CORES
    k_of_w = rank_of_w % K_CORES
    cnt_kj = np.zeros((K_CORES, SPC), np.int64)
    cnt_kj[k_of_w, j_of_w] = counts
    cap = ((cnt_kj.max(axis=0) + P - 1) // P)      # chunks per slot (shared)
    C = int(cap.sum())
    T = C * P
    slot_base = np.zeros(SPC, np.int64)
    slot_base[1:] = np.cumsum(cap * P)[:-1]

    # position of each sorted edge within its core's padded [T] array
    pos = slot_base[j_of_w[win]] + (np.arange(E) - wstart[win])
    core_of_edge = k_of_w[win]

    # int8 quantization with per-edge scales (value rows and w rows)
    vabs = np.maximum(np.abs(value).max(axis=1), 1e-30)
    vscale = (vabs / 127.0).astype(np.float32)
    q = np.clip(np.rint(value * (1.0 / vscale)[:, None]), -127, 127).astype(np.int8)
    w_full = (cutoff[:, None] * edge_weights).astype(np.float32)
    wabs = np.maximum(np.abs(w_full).max(axis=1), 1e-30)
    wscale = (wabs / 127.0).astype(np.float32)
    w8_full = np.clip(np.rint(w_full * (1.0 / wscale)[:, None]),
                      -127, 127).astype(np.int8)

    def to_pc(a):  # [T, ...] -> [128, C, ...] with slot t -> (t % 128, t // 128)
        return np.ascontiguousarray(
            a.reshape((C, P) + a.shape[1:]).swapaxes(0, 1))

    in_maps = []
    for k in range(K_CORES):
        m = core_of_edge == k
        pk, srck = pos[m], order[m]
        pv = np.zeros((T, VCOLS), np.int8)
        pv[pk] = q[srck]
        w8 = np.full((T, NUM_HEADS), -128, np.int8)   # padding: w8*wscale=-30080
        w8[pk] = w8_full[srck]
        aux = np.zeros((T, AUXC), np.float16)
        aux[:, 0] = 235.0                             # padding wscale
        aux[pk, 0] = wscale[srck]
        aux[pk, 1] = vscale[srck]
        aux[pk, 2] = (dsts[m] & 127).astype(np.float16)
        in_maps.append({"pv": to_pc(pv), "w8": to_pc(w8), "aux": to_pc(aux)})

    nc = _build(cap)
    last_nc, last_in_maps = nc, in_maps
    res = run_bass_kernel_spmd(
        nc, in_maps, core_ids=list(range(K_CORES)),
        trace=_trace, **(_trace_kwargs or {}))
    last_results = res

    # ---- unshard: node n lives at core k_of_w[n>>7], row (j_of_w[n>>7], n&127)
    out = np.zeros((N_NODES, VCOLS), np.float32)
    nodes = np.arange(N_NODES)
    w_of_n = nodes >> 7
    for k in range(K_CORES):
        o8 = res.results[k]["o8"]                   # [128, SPC, 320] int8
        nsc = res.results[k]["nsc"].astype(np.float32)  # [128, SPC]
        m = k_of_w[w_of_n] == k
        nk = nodes[m]
        p_idx = nk & 127
        j_idx = j_of_w[w_of_n[m]]
        out[nk] = o8[p_idx, j_idx, :].astype(np.float32) * nsc[p_idx, j_idx, None]
    return out
